# revision 2
# baseline (speedup 1.0000x reference)
"""MoE feed-forward (E=8 experts, top-2 routing) on 8 Trainium2 NeuronCores.

Strategy: expert-parallel dispatch with a 3-slot uniform SPMD structure.
Host computes the (cheap) routing exactly as the reference does, then
bin-packs the per-expert token lists into a fixed per-core tile
structure, default [512, 384, 256] = 1152 slots/core (vs 8192/8 = 1024
ideal).  Each of the 3 tiles has its own expert weight set (chosen per
core by the host), so any mix of expert loads packs with <= 12.5%
padding; a DP solver picks per-expert bin counts (8 bins per slot
globally), falling back to wider structures if infeasible.  All tiles
are >= 256 wide, so every matmul streams enough columns to hide
LDWEIGHTS and run the PE at full rate.

Device kernel (SPMD, same program all 8 cores): per tile t with that
tile's expert weights streamed from HBM,
    hT  = silu(Wg^T x + bg) * (W1^T x + b1)      [feature-major, [H, tok]]
    out = gate * (W2^T hT + b2)                  [[D, tok], bf16]
All matmuls use natural weight layouts as the stationary operand
(activations stay feature-major; no transposes), bf16 inputs with fp32
PSUM accumulation.  Matmul accumulation groups are kept contiguous;
PSUM slots rotate through all 8 banks via one shared pool.  Weight
DMAs own the SP DGE queue; x/gate/bias DMAs ride the Activation DGE
queue so the first matmul can start ~1us in (input and weight streams
are issued by different engines and overlap).
"""

import numpy as np

E = 8
K = 2
D = 1024
H = 2736
B, S = 2, 2048
T = B * S
N_CORES = 8
P = 128
DC = D // P            # 8 d-chunks
HC = (H + P - 1) // P  # 22 h-chunks
HP = HC * P            # 2816 padded hidden
NSLOT = 3
BLKW = 512             # PSUM tile width (one full bank)

# candidate per-core tile structures, tried in order
STRUCTURES = [
    (512, 384, 256),
    (512, 512, 256),
    (512, 512, 384),
    (512, 512, 512),   # always feasible: sum ceil(t_e/512) <= 16+8
]

_prog_cache: dict = {}


def _bf16(a):
    import ml_dtypes
    return np.ascontiguousarray(a.astype(ml_dtypes.bfloat16))


def _routing(x, centroid):
    """Mirror reference.py's routing math op-for-op (same platform => same
    top-k selection and softmax weights)."""
    import jax
    import jax.numpy as jnp
    xs = jnp.asarray(x, jnp.float32).reshape(T, D)
    c = jnp.asarray(centroid, jnp.float32)
    d2 = (jnp.sum(xs * xs, axis=-1, keepdims=True)
          + jnp.sum(c * c, axis=-1)[None, :]
          - 2.0 * (xs @ c.T))
    dist = jnp.sqrt(jnp.maximum(d2, 1e-12))
    w, sel = jax.lax.top_k(dist, K)
    w = jax.nn.softmax(w.astype(jnp.float32), axis=1)
    return np.asarray(sel), np.asarray(w, np.float32)


def _solve_bins(counts, widths):
    """Pick per-expert bin usage (n_s bins from slot s, 8 bins per slot)
    covering each expert's token count; minimize total padding.
    Returns list of per-expert tuples, or None if infeasible."""
    cands = []
    for t in counts:
        opts = []
        for n1 in range(9):
            for n2 in range(9):
                for n3 in range(9):
                    cap = n1 * widths[0] + n2 * widths[1] + n3 * widths[2]
                    if t == 0:
                        if (n1, n2, n3) == (0, 0, 0):
                            opts.append((0, n1, n2, n3))
                        continue
                    if cap >= t:
                        opts.append((cap - t, n1, n2, n3))
        opts.sort()
        pruned = []
        for pad, n1, n2, n3 in opts:
            if not any(p1 <= n1 and p2 <= n2 and p3 <= n3
                       for _, p1, p2, p3 in pruned):
                pruned.append((pad, n1, n2, n3))
        cands.append(pruned)

    best = {(0, 0, 0): (0, [])}
    for opts in cands:
        nxt = {}
        for (u1, u2, u3), (pad0, chs) in best.items():
            for pad, n1, n2, n3 in opts:
                s = (u1 + n1, u2 + n2, u3 + n3)
                if s[0] > 8 or s[1] > 8 or s[2] > 8:
                    continue
                tot = pad0 + pad
                if s not in nxt or nxt[s][0] > tot:
                    nxt[s] = (tot, chs + [(n1, n2, n3)])
        best = nxt
        if not best:
            return None
    return min(best.values())[1]


def _build_plan(sel, w):
    """Dispatch: pack per-expert token lists into the per-core tile
    structure.  Returns (widths, sets, slot_tok, slot_gate, slot_exp)."""
    tok_e, gate_e = {}, {}
    counts = []
    for e in range(E):
        tks, ks = np.nonzero(sel == e)
        tok_e[e] = tks
        gate_e[e] = w[tks, ks]
        counts.append(len(tks))

    for widths in STRUCTURES:
        sol = _solve_bins(counts, widths)
        if sol is not None:
            break
    assert sol is not None, f"no feasible structure for counts {counts}"

    # build global bin lists per slot: (expert, tok_start, fill)
    slot_bins = [[] for _ in range(NSLOT)]
    for e, (n1, n2, n3) in enumerate(sol):
        pos = 0
        for s, n in enumerate((n1, n2, n3)):
            for _ in range(n):
                fill = min(widths[s], counts[e] - pos)
                slot_bins[s].append((e, pos, fill))
                pos += fill
        assert pos >= counts[e], (e, pos, counts[e], sol[e])
    for s in range(NSLOT):
        assert len(slot_bins[s]) <= N_CORES
        while len(slot_bins[s]) < N_CORES:
            slot_bins[s].append((0, 0, 0))  # dummy: gates all zero

    sets, slot_tok, slot_gate, slot_exp = [], [], [], []
    for c in range(N_CORES):
        es, toks, gates, exps = [], [], [], []
        for s in range(NSLOT):
            e, pos, fill = slot_bins[s][c]
            wsl = widths[s]
            tt = np.zeros(wsl, np.int64)
            gg = np.zeros(wsl, np.float32)
            tt[:fill] = tok_e[e][pos:pos + fill]
            gg[:fill] = gate_e[e][pos:pos + fill]
            es.append(e)
            toks.append(tt)
            gates.append(gg)
            exps.append(np.full(wsl, e, np.int64))
        sets.append(tuple(es))
        slot_tok.append(np.concatenate(toks))
        slot_gate.append(np.concatenate(gates))
        slot_exp.append(np.concatenate(exps))
    return widths, sets, slot_tok, slot_gate, slot_exp


def _build_program(widths=(512, 384, 256), reps=1):
    """Build + compile the SPMD Bass program for the given tile widths."""
    import sys
    if "/opt/trn_rl_repo" not in sys.path:
        sys.path.insert(0, "/opt/trn_rl_repo")
    import concourse.bacc as bacc
    import concourse.bass as bass
    import concourse.tile as tile
    from concourse import mybir

    f32 = mybir.dt.float32
    bf16 = mybir.dt.bfloat16
    AF = mybir.ActivationFunctionType
    OP = mybir.AluOpType

    CAP = sum(widths)
    offs = [0]
    for wd in widths:
        offs.append(offs[-1] + wd)

    nc = bacc.Bacc("TRN2", target_bir_lowering=False, num_devices=N_CORES)
    xt_d = nc.dram_tensor("xt", [P, DC, CAP], bf16, kind="ExternalInput")
    g_d = nc.dram_tensor("gates", [1, CAP], f32, kind="ExternalInput")
    wg_d = nc.dram_tensor("wg", [NSLOT, HC, P, DC, P], bf16,
                          kind="ExternalInput")
    w1_d = nc.dram_tensor("w1", [NSLOT, HC, P, DC, P], bf16,
                          kind="ExternalInput")
    w2_d = nc.dram_tensor("w2", [NSLOT, P, HC, DC, P], bf16,
                          kind="ExternalInput")
    bg_d = nc.dram_tensor("bg", [P, NSLOT * HC], f32, kind="ExternalInput")
    b1_d = nc.dram_tensor("b1", [P, NSLOT * HC], f32, kind="ExternalInput")
    b2_d = nc.dram_tensor("b2", [P, NSLOT * DC], f32, kind="ExternalInput")
    out_d = nc.dram_tensor("out", [DC, P, CAP], bf16, kind="ExternalOutput")

    with tile.TileContext(nc) as tc:
        with (
            tc.tile_pool(name="xp", bufs=1) as xp,
            tc.tile_pool(name="gp", bufs=1) as gp,
            tc.tile_pool(name="bp", bufs=1) as bp,
            tc.tile_pool(name="wgp", bufs=10) as wgp,
            tc.tile_pool(name="w1p", bufs=10) as w1p,
            tc.tile_pool(name="w2cp", bufs=8) as w2cp,
            tc.tile_pool(name="hp", bufs=2) as hp,
            tc.tile_pool(name="sgp", bufs=3) as sgp,
            tc.tile_pool(name="op", bufs=4) as op_,
            tc.tile_pool(name="pp", bufs=8, space="PSUM") as pp,
        ):
            # x / gate / bias DMAs ride the Activation DGE queue so the SP
            # queue belongs to the weight stream; first matmul needs only
            # wg[t0,h0] (SP) + xt tile0 chunk d0 (Activation), ~1us in.
            xt = xp.tile([P, DC, CAP], bf16)
            for t in range(NSLOT):
                for d in range(DC):
                    nc.scalar.dma_start(
                        out=xt[:, d, offs[t]:offs[t + 1]],
                        in_=xt_d[:, d, offs[t]:offs[t + 1]])
                if t == 0:
                    bg = bp.tile([P, NSLOT * HC], f32, tag="bg")
                    b1 = bp.tile([P, NSLOT * HC], f32, tag="b1")
                    nc.scalar.dma_start(out=bg[:], in_=bg_d[:])
                    nc.scalar.dma_start(out=b1[:], in_=b1_d[:])
            gate = gp.tile([P, CAP], f32)
            g_ap = g_d[:]
            nc.scalar.dma_start(
                out=gate[:],
                in_=bass.AP(tensor=g_ap.tensor, offset=g_ap.offset,
                            ap=[[0, P], [1, CAP]]))
            b2 = bp.tile([P, NSLOT * DC], f32, tag="b2")
            nc.scalar.dma_start(out=b2[:], in_=b2_d[:])

            def phase1(t, hts):
                w = widths[t]
                off = offs[t]
                for h in range(HC):
                    wgt = wgp.tile([P, DC, P], bf16)
                    nc.sync.dma_start(out=wgt[:], in_=wg_d[t, h])
                    w1t = w1p.tile([P, DC, P], bf16)
                    nc.sync.dma_start(out=w1t[:], in_=w1_d[t, h])
                    pg = pp.tile([P, BLKW], f32, tag="ps", name="pg")
                    p1 = pp.tile([P, BLKW], f32, tag="ps", name="p1")
                    for d in range(DC):
                        nc.tensor.matmul(
                            pg[:, :w], wgt[:, d, :], xt[:, d, off:off + w],
                            start=(d == 0), stop=(d == DC - 1))
                    for d in range(DC):
                        nc.tensor.matmul(
                            p1[:, :w], w1t[:, d, :], xt[:, d, off:off + w],
                            start=(d == 0), stop=(d == DC - 1))
                    sg = sgp.tile([P, BLKW], f32)
                    nc.scalar.activation(
                        out=sg[:, :w], in_=pg[:, :w], func=AF.Silu,
                        bias=bg[:, t * HC + h:t * HC + h + 1], scale=1.0)
                    nc.vector.scalar_tensor_tensor(
                        out=hts[:, h, :w], in0=p1[:, :w],
                        scalar=b1[:, t * HC + h:t * HC + h + 1],
                        in1=sg[:, :w], op0=OP.add, op1=OP.mult)

            def phase2(t, hts):
                w = widths[t]
                off = offs[t]
                pos = [pp.tile([P, BLKW], f32, tag="ps", name="po")
                       for _ in range(DC)]
                for h in range(HC):
                    w2c = w2cp.tile([P, DC, P], bf16)
                    nc.sync.dma_start(out=w2c[:], in_=w2_d[t, :, h])
                    for d in range(DC):
                        nc.tensor.matmul(
                            pos[d][:, :w], w2c[:, d, :], hts[:, h, :w],
                            start=(h == 0), stop=(h == HC - 1))
                for d in range(DC):
                    osb = op_.tile([P, BLKW], bf16)
                    nc.vector.scalar_tensor_tensor(
                        out=osb[:, :w], in0=pos[d][:, :w],
                        scalar=b2[:, t * DC + d:t * DC + d + 1],
                        in1=gate[:, off:off + w],
                        op0=OP.add, op1=OP.mult)
                    nc.sync.dma_start(out=out_d[d, :, off:off + w],
                                      in_=osb[:, :w])

            for _rep in range(reps):
                for t in range(NSLOT):
                    hts = hp.tile([P, HC, BLKW], bf16, tag="ht")
                    phase1(t, hts)
                    phase2(t, hts)
    nc.compile()
    return nc


def _pack_core_inputs(widths, sets_c, slot_tok_c, slot_gate_c, xs,
                      wg_pe, w1_pe, w2_pe, bg_pe, b1_pe, b2_pe):
    CAP = sum(widths)
    x_slots = xs[slot_tok_c]                        # [CAP, D] f32
    xt = np.ascontiguousarray(
        x_slots.T.reshape(DC, P, CAP).transpose(1, 0, 2))  # [P, DC, CAP]
    return {
        "xt": _bf16(xt),
        "gates": slot_gate_c.reshape(1, CAP),
        "wg": np.stack([wg_pe[e] for e in sets_c]),
        "w1": np.stack([w1_pe[e] for e in sets_c]),
        "w2": np.stack([w2_pe[e] for e in sets_c]),
        "bg": np.concatenate([bg_pe[e] for e in sets_c], 1),
        "b1": np.concatenate([b1_pe[e] for e in sets_c], 1),
        "b2": np.concatenate([b2_pe[e] for e in sets_c], 1),
    }


def prepare(x, centroid, Wg, bg, W1, b1, W2, b2, reps=1):
    """Host side: routing + dispatch. Returns (nc, in_maps, plan)."""
    x = np.asarray(x, np.float32)
    centroid = np.asarray(centroid, np.float32)
    Wg = np.asarray(Wg, np.float32)
    W1 = np.asarray(W1, np.float32)
    W2 = np.asarray(W2, np.float32)
    bg = np.asarray(bg, np.float32)
    b1 = np.asarray(b1, np.float32)
    b2 = np.asarray(b2, np.float32)

    sel, w = _routing(x, centroid)
    widths, sets, slot_tok, slot_gate, slot_exp = _build_plan(sel, w)

    key = (widths, reps)
    if key not in _prog_cache:
        _prog_cache[key] = _build_program(widths=widths, reps=reps)
    nc = _prog_cache[key]

    WgP = np.zeros((E, D, HP), np.float32)
    WgP[:, :, :H] = Wg
    W1P = np.zeros((E, D, HP), np.float32)
    W1P[:, :, :H] = W1
    W2P = np.zeros((E, HP, D), np.float32)
    W2P[:, :H, :] = W2
    bgP = np.zeros((E, HP), np.float32)
    bgP[:, :H] = bg
    b1P = np.zeros((E, HP), np.float32)
    b1P[:, :H] = b1
    # [h, p, d, c] layouts
    wg_pe = [_bf16(WgP[e].reshape(DC, P, HC, P).transpose(2, 1, 0, 3))
             for e in range(E)]
    w1_pe = [_bf16(W1P[e].reshape(DC, P, HC, P).transpose(2, 1, 0, 3))
             for e in range(E)]
    # [p, h, d, c] layout
    w2_pe = [_bf16(W2P[e].reshape(HC, P, DC, P).transpose(1, 0, 2, 3))
             for e in range(E)]
    bg_pe = [np.ascontiguousarray(bgP[e].reshape(HC, P).T) for e in range(E)]
    b1_pe = [np.ascontiguousarray(b1P[e].reshape(HC, P).T) for e in range(E)]
    b2_pe = [np.ascontiguousarray(b2[e].reshape(DC, P).T) for e in range(E)]

    xs = x.reshape(T, D)
    in_maps = [
        _pack_core_inputs(widths, sets[c], slot_tok[c], slot_gate[c], xs,
                          wg_pe, w1_pe, w2_pe, bg_pe, b1_pe, b2_pe)
        for c in range(N_CORES)
    ]
    plan = (slot_tok, slot_gate, slot_exp)
    return nc, in_maps, plan


def combine(results, plan):
    """Scatter-add per-core outputs back to the full [B, S, D] output."""
    slot_tok, slot_gate, slot_exp = plan
    out = np.zeros((T, D), np.float32)
    for e in range(E):
        idxs, vals = [], []
        for c in range(N_CORES):
            ovals = results[c]["out"]  # [DC, P, CAP] bf16
            m = (slot_exp[c] == e) & (slot_gate[c] != 0.0)
            if not m.any():
                continue
            sl = np.nonzero(m)[0]
            idxs.append(slot_tok[c][sl])
            cap = ovals.shape[-1]
            vals.append(np.asarray(ovals, np.float32)
                        .reshape(D, cap)[:, sl].T)  # [n, D]
        if not idxs:
            continue
        idx = np.concatenate(idxs)
        val = np.concatenate(vals)
        # token indices are unique within one expert
        out[idx] += val
    return out.reshape(B, S, D)


def kernel(x, centroid, Wg, bg, W1, b1, W2, b2):
    import sys
    if "/opt/trn_rl_repo" not in sys.path:
        sys.path.insert(0, "/opt/trn_rl_repo")
    from concourse.bass_utils import run_bass_kernel_spmd

    nc, in_maps, plan = prepare(x, centroid, Wg, bg, W1, b1, W2, b2)
    res = run_bass_kernel_spmd(nc, in_maps, list(range(N_CORES)))
    return combine(res.results, plan)
